# revision 13
# baseline (speedup 1.0000x reference)
"""Trainium2 Bass kernel for nn_MoEALU (soft ripple-carry byte adder).

Math (equivalent to reference, exploiting table structure):
  - b2n contraction == segmented sums of the 256-byte distribution
    (high nibble: 16 contiguous groups; low nibble: stride-16 groups).
  - add_table/carry_table contraction of w = x (x) y (x) cin decomposes via the
    linear convolution z = conv(x, y) (z[31]==0 pad):
        u[m]  = z[m] + z[m+16]            (mod-16 fold)
        s     = u*c0 + rot1(u)*c1         (soft sum logits)
        cr0   = Z0*c0 + (Z0 - z15)*c1     (Z0 = sum z[0:16])
        cr1   = Z1*c0 + (Z1 + z15)*c1     (Z1 = sum z[16:32])
  - n2b contraction == broadcast add: o[i,j] = sh[i] + sl[j].
Softmaxes: softmax1 uses true max; chain softmaxes use fixed offset
exp(100*v - 100) (safe: max component >= 1/16); output softmax uses the exact
max Mh+Ml as per-partition activation bias.

Sharding: pure data parallel over batch, 8 cores x 4096 rows.
"""

import numpy as np

B_FULL = 32768
N_CORES = 8
B_CORE = B_FULL // N_CORES  # 4096
P = 128
NT = B_CORE // P  # 32 tiles

_BUILT = None


def _build():
    import concourse.bass as bass
    import concourse.bacc as bacc
    import concourse.mybir as mybir
    import concourse.tile as tile

    f32 = mybir.dt.float32
    AF = mybir.ActivationFunctionType
    AX = mybir.AxisListType
    OP = mybir.AluOpType

    nc = bacc.Bacc("TRN2", target_bir_lowering=False, debug=False)
    a_d = nc.dram_tensor("a", [B_CORE, 4, 256], f32, kind="ExternalInput")
    b_d = nc.dram_tensor("b", [B_CORE, 4, 256], f32, kind="ExternalInput")
    out_d = nc.dram_tensor("out", [B_CORE, 4, 256], f32, kind="ExternalOutput")

    def rawap(base_ap, off_elems, dims):
        # dims: list of [step, count] free dims; keeps base partition dim
        part = base_ap.ap[0]
        return bass.AP(base_ap.tensor, base_ap.offset + off_elems, [list(part)] + [list(d) for d in dims])

    with tile.TileContext(nc) as tc:
        with (
            tc.tile_pool(name="persist", bufs=1) as pp,
            tc.tile_pool(name="pin", bufs=3) as pin,
            tc.tile_pool(name="pmid", bufs=2) as pmid,
            tc.tile_pool(name="psm", bufs=2) as psm,
            tc.tile_pool(name="pc", bufs=2) as pc,
            tc.tile_pool(name="pcs", bufs=2) as pcs,
        ):
            # ---- persistent tensors ----
            ucat = pp.tile([P, 8, 2, 32, 18], f32, tag="ucat")  # [stage, half, blk, 18]
            A_all = pp.tile([P, 8, 32, 16], f32, tag="A_all")
            carry = pp.tile([P, 32, 2], f32, tag="carry")
            sgn = pp.tile([P, 2], f32, tag="sgn")
            nb100 = pp.tile([P, 1], f32, tag="nb100")
            w_t0 = pp.tile([P, 8, 16, 32], f32, tag="w0")
            w_t1 = pp.tile([P, 8, 16, 32], f32, tag="w1")
            z_t0 = pp.tile([P, 8, 32], f32, tag="z0")
            z_t1 = pp.tile([P, 8, 32], f32, tag="z1")
            w_bufs = [w_t0, w_t1]
            z_bufs = [z_t0, z_t1]
            # chain scratch (serial reuse)
            P_t = pp.tile([P, 2, 32, 18], f32, tag="P_t")
            st_t = pp.tile([P, 32, 18], f32, tag="st")
            e_t = pp.tile([P, 32, 18], f32, tag="e")
            ns_t = pp.tile([P, 32], f32, tag="ns")
            ncr_t = pp.tile([P, 32], f32, tag="ncr")
            rs_t = pp.tile([P, 32], f32, tag="rs")
            rc_t = pp.tile([P, 32], f32, tag="rc")

            nc.gpsimd.memset(nb100[:], -100.0)
            nc.gpsimd.memset(sgn[:, 0:1], -1.0)
            nc.gpsimd.memset(sgn[:, 1:2], 1.0)
            for k in range(2):
                nc.gpsimd.memset(w_bufs[k][:, :, :, 16:32], 0.0)
                nc.gpsimd.memset(z_bufs[k][:, :, 31:32], 0.0)
            nc.gpsimd.memset(carry[:, :, 0:1], 1.0)
            nc.gpsimd.memset(carry[:, :, 1:2], 0.0)

            a_v = a_d.ap().rearrange("(n p) f g -> n p (f g)", p=P)
            b_v = b_d.ap().rearrange("(n p) f g -> n p (f g)", p=P)
            o_v = out_d.ap().rearrange("(n p) f g -> n p (f g)", p=P)

            # ================= Phase A: per-tile nibble dists + conv =========
            for i in range(NT):
                w_t = w_bufs[i % 2]
                z_t = z_bufs[i % 2]
                a_t = pin.tile([P, 1024], f32, tag="a")
                b_t = pin.tile([P, 1024], f32, tag="b")
                nc.sync.dma_start(a_t[:], a_v[i])
                nc.sync.dma_start(b_t[:], b_v[i])

                # nibble sums -> c_all [16 groups x 16]; group order:
                # a: (lo p0, hi p0, lo p1, hi p1, ...) = groups 0..7, b: groups 8..15
                c_all = pmid.tile([P, 256], f32, tag="c_all")
                for src, base in ((a_t, 0), (b_t, 128)):
                    hi_in = src[:].rearrange("p (x h l) -> p x h l", x=4, h=16, l=16)
                    # high: sum over low nibble (contiguous innermost)
                    nc.vector.tensor_reduce(
                        rawap(c_all[:], base + 16, [[32, 4], [1, 16]]),
                        hi_in, axis=AX.X, op=OP.add)
                    # low: sum over high nibble (innermost step 16)
                    lo_in = rawap(src[:], 0, [[256, 4], [1, 16], [16, 16]])
                    nc.vector.tensor_reduce(
                        rawap(c_all[:], base + 0, [[32, 4], [1, 16]]),
                        lo_in, axis=AX.X, op=OP.add)

                # softmax over each of the 16 groups
                m16 = psm.tile([P, 16], f32, tag="m16")
                cg = c_all[:].rearrange("p (g e) -> p g e", g=16)
                nc.vector.tensor_reduce(m16[:], cg, axis=AX.X, op=OP.max)
                ts = pmid.tile([P, 256], f32, tag="tsub")
                nc.vector.tensor_sub(
                    ts[:].rearrange("p (g e) -> p g e", g=16), cg,
                    m16[:].unsqueeze(2).broadcast_to([P, 16, 16]))
                te = pmid.tile([P, 256], f32, tag="texp")
                nc.scalar.activation(te[:], ts[:], AF.Exp, scale=100.0)
                n16 = psm.tile([P, 16], f32, tag="n16")
                nc.vector.tensor_reduce(
                    n16[:], te[:].rearrange("p (g e) -> p g e", g=16),
                    axis=AX.X, op=OP.add)
                r16 = psm.tile([P, 16], f32, tag="r16")
                nc.vector.reciprocal(r16[:], n16[:])
                x_t = pmid.tile([P, 256], f32, tag="x_t")
                nc.vector.tensor_mul(
                    x_t[:].rearrange("p (g e) -> p g e", g=16),
                    te[:].rearrange("p (g e) -> p g e", g=16),
                    r16[:].unsqueeze(2).broadcast_to([P, 16, 16]))

                # outer products w[s,i,j] = xa[s,i]*xb[s,j] (cols 0..15; 16..31 stay 0)
                xa = rawap(x_t[:], 0, [[16, 8], [1, 16], [0, 16]])
                xb = rawap(x_t[:], 128, [[16, 8], [0, 16], [1, 16]])
                nc.gpsimd.tensor_mul(w_t[:, :, :, 0:16], xa, xb)

                # z[s,t] = sum_i w[s, i, t-i]  (antidiagonal, flat stride 31)
                nc.vector.tensor_reduce(
                    rawap(z_t[:], 0, [[32, 8], [1, 31]]),
                    rawap(w_t[:], 0, [[512, 8], [1, 31], [31, 16]]),
                    axis=AX.X, op=OP.add)

                # zz[s, half] = sum z[s, 16*half : 16*half+16]
                zz = psm.tile([P, 16], f32, tag="zz")
                nc.vector.tensor_reduce(
                    zz[:].rearrange("p (s h) -> p s h", s=8),
                    z_t[:].rearrange("p s (h e) -> p s h e", h=2),
                    axis=AX.X, op=OP.add)

                # u = z[:,0:16] + z[:,16:32] -> ucat[:, s, 0, i, 0:16]
                nc.vector.tensor_add(
                    rawap(ucat[:], 18 * i, [[1152, 8], [1, 16]]),
                    z_t[:, :, 0:16], z_t[:, :, 16:32])
                # rot half: rot[l] = u[l-1]; rot[0] = u[15]  (ACT copies: DVE is scarce)
                nc.scalar.copy(
                    rawap(ucat[:], 576 + 18 * i + 1, [[1152, 8], [1, 15]]),
                    rawap(ucat[:], 18 * i, [[1152, 8], [1, 15]]))
                nc.scalar.copy(
                    rawap(ucat[:], 576 + 18 * i, [[1152, 8], [1, 1]]),
                    rawap(ucat[:], 18 * i + 15, [[1152, 8], [1, 1]]))
                # main extras: [Z0, Z1]
                nc.scalar.copy(
                    rawap(ucat[:], 18 * i + 16, [[1152, 8], [1, 2]]),
                    zz[:].rearrange("p (s h) -> p s h", s=8))
                # rot extras: [Z0 - z15, Z1 + z15] = zz + z15*[-1, +1]
                zs = psm.tile([P, 16], f32, tag="zs")
                nc.vector.tensor_mul(
                    zs[:].rearrange("p (s h) -> p s h", s=8),
                    rawap(z_t[:], 15, [[32, 8], [0, 2]]),
                    sgn[:].unsqueeze(1).broadcast_to([P, 8, 2]))
                nc.vector.tensor_add(
                    rawap(ucat[:], 576 + 18 * i + 16, [[1152, 8], [1, 2]]),
                    zz[:].rearrange("p (s h) -> p s h", s=8),
                    zs[:].rearrange("p (s h) -> p s h", s=8))

            # ================= Phase B: serial carry chain ===================
            for s in range(8):
                # P = ucat[s] * carry  (main*c0, rot*c1)
                nc.vector.tensor_mul(
                    P_t[:], ucat[:, s],
                    rawap(carry[:], 0, [[1, 2], [2, 32], [0, 18]]))
                nc.vector.tensor_add(st_t[:], P_t[:, 0], P_t[:, 1])
                nc.scalar.activation(e_t[:], st_t[:], AF.Exp, bias=nb100[:], scale=100.0)
                nc.vector.tensor_reduce(ns_t[:], e_t[:, :, 0:16], axis=AX.X, op=OP.add)
                nc.vector.tensor_reduce(ncr_t[:], e_t[:, :, 16:18], axis=AX.X, op=OP.add)
                nc.vector.reciprocal(rs_t[:], ns_t[:])
                nc.vector.reciprocal(rc_t[:], ncr_t[:])
                # new carry
                nc.vector.tensor_mul(
                    rawap(carry[:], 0, [[2, 32], [1, 2]]),
                    e_t[:, :, 16:18],
                    rc_t[:].unsqueeze(2).broadcast_to([P, 32, 2]))
                # normalized s-dist
                nc.vector.tensor_mul(
                    A_all[:, s], e_t[:, :, 0:16],
                    rs_t[:].unsqueeze(2).broadcast_to([P, 32, 16]))

            # ================= Phase C: output softmax =======================
            for i in range(NT):
                M8 = pcs.tile([P, 8], f32, tag="M8")
                nc.vector.tensor_reduce(
                    M8[:],
                    rawap(A_all[:], 16 * i, [[512, 8], [1, 16]]),
                    axis=AX.X, op=OP.max)
                Ms4 = pcs.tile([P, 4], f32, tag="Ms4")
                nc.vector.tensor_add(
                    Ms4[:],
                    rawap(M8[:], 0, [[2, 4]]),
                    rawap(M8[:], 1, [[2, 4]]))
                nb4 = pcs.tile([P, 4], f32, tag="nb4")
                nc.vector.tensor_scalar_mul(nb4[:], Ms4[:], -100.0)
                o_t = pc.tile([P, 4, 16, 16], f32, tag="o_t")
                # o[p, pos, ih, jl] = sh[pos, ih] + sl[pos, jl]  (on GpSimd: DVE is scarce)
                nc.gpsimd.tensor_add(
                    o_t[:],
                    rawap(A_all[:], 512 + 16 * i, [[1024, 4], [1, 16], [0, 16]]),
                    rawap(A_all[:], 16 * i, [[1024, 4], [0, 16], [1, 16]]))
                no4 = pcs.tile([P, 4], f32, tag="no4")
                for p4 in range(4):
                    nc.scalar.activation(
                        o_t[:, p4], o_t[:, p4], AF.Exp,
                        bias=nb4[:, p4:p4 + 1], scale=100.0,
                        accum_out=no4[:, p4:p4 + 1])
                ro4 = pcs.tile([P, 4], f32, tag="ro4")
                nc.vector.reciprocal(ro4[:], no4[:])
                for p4 in range(4):
                    nc.vector.tensor_scalar_mul(o_t[:, p4], o_t[:, p4], ro4[:, p4:p4 + 1])
                nc.sync.dma_start(o_v[i], o_t[:].rearrange("p a b c -> p (a b c)"))

    nc.compile()
    return nc


def _get_nc():
    global _BUILT
    if _BUILT is None:
        _BUILT = _build()
    return _BUILT


def kernel(a, b, add_table=None, carry_table=None, b2n=None, n2b=None, **_kw):
    from concourse.bass_utils import run_bass_kernel_spmd

    a = np.ascontiguousarray(np.asarray(a, dtype=np.float32))
    b = np.ascontiguousarray(np.asarray(b, dtype=np.float32))
    nc = _get_nc()
    in_maps = [
        {"a": a[i * B_CORE:(i + 1) * B_CORE], "b": b[i * B_CORE:(i + 1) * B_CORE]}
        for i in range(N_CORES)
    ]
    res = run_bass_kernel_spmd(nc, in_maps, core_ids=list(range(N_CORES)))
    out = np.concatenate([r["out"] for r in res.results], axis=0)
    return out.astype(np.float32)
